# revision 12
# baseline (speedup 1.0000x reference)
"""Trainium2 Bass kernel for nn_CLoss_inout: mean(1 - rowwise_dot(A, B)).

Full inputs A, B are [1048576, 128] f32. result = 1 - sum(A*B)/N (or
mean(A*B)+1 when flip). Data-parallel over 8 NeuronCores: core c gets rows
[c*131072, (c+1)*131072), viewed as [128 partitions x 131072 free] (order
of summation is irrelevant). Per tile of [128 x FT]: two HWDGE DMA loads,
one DVE tensor_mul (f32 inputs, bf16 product), and FT/512 PE matmuls
against a ones[128,1] stationary vector that accumulate per-column sums
into a single PSUM bank across the whole kernel. Tail: PSUM -> SBUF copy,
DMA the [1,512] partial out. The 8 per-core partials are summed on host.
"""

import numpy as np

N, D = 1048576, 128
M = 8                     # cores
ROWS = N // M             # 131072 rows per core
P = 128                   # SBUF partitions
FREE = ROWS * D // P      # 131072 f32 per partition per tensor
FT = 8192                 # tile free size: 128 x 8192 f32 = 4 MiB per DMA
BUFS = 2
MMF = 512                 # matmul moving free dim (one PSUM bank of f32)

TRACE = False             # test.py sets True to capture an NTFF profile
LAST = {}                 # stash of the most recent BassKernelResults

_cache = {}


def _ensure_path():
    import sys
    try:
        import concourse.bass  # noqa: F401
    except ImportError:
        sys.path.insert(0, "/opt/trn_rl_repo")


def build(free=FREE, ft=FT, bufs=BUFS):
    _ensure_path()
    import concourse.bacc as bacc
    import concourse.mybir as mybir
    from concourse.tile import TileContext

    assert free % ft == 0 and ft % MMF == 0
    nt = free // ft
    nj = ft // MMF
    # Bacc (not raw Bass): its compile pipeline splits multi-wait
    # instructions (TRN2 allows at most one sync wait per instruction).
    nc = bacc.Bacc(None, name="closs_inout")
    a = nc.dram_tensor("input_in", [P, free], mybir.dt.float32, kind="ExternalInput")
    b = nc.dram_tensor("input_out", [P, free], mybir.dt.float32, kind="ExternalInput")
    o = nc.dram_tensor("partial", [1, MMF], mybir.dt.float32, kind="ExternalOutput")

    with TileContext(nc) as tc:
        with (
            tc.tile_pool(name="pa", bufs=bufs) as pa,
            tc.tile_pool(name="pb", bufs=bufs) as pb,
            tc.tile_pool(name="pp", bufs=bufs) as pp,
            tc.tile_pool(name="misc", bufs=1) as misc,
            tc.tile_pool(name="psum", bufs=1, space="PSUM") as psum,
        ):
            ones = misc.tile([P, 1], mybir.dt.bfloat16)
            nc.gpsimd.memset(ones[:], 1.0)
            ps = psum.tile([1, MMF], mybir.dt.float32)
            for i in range(nt):
                at = pa.tile([P, ft], mybir.dt.float32)
                bt = pb.tile([P, ft], mybir.dt.float32)
                nc.sync.dma_start(out=at[:], in_=a[:, i * ft:(i + 1) * ft])
                nc.sync.dma_start(out=bt[:], in_=b[:, i * ft:(i + 1) * ft])
                pt = pp.tile([P, ft], mybir.dt.bfloat16)
                nc.vector.tensor_mul(pt[:], at[:], bt[:])
                for j in range(nj):
                    # ps[0, n] += sum_p pt[p, j*MMF + n]
                    nc.tensor.matmul(
                        ps[:, :],
                        ones[:],
                        pt[:, j * MMF:(j + 1) * MMF],
                        start=(i == 0 and j == 0),
                        stop=(i == nt - 1 and j == nj - 1),
                    )
            out_sb = misc.tile([1, MMF], mybir.dt.float32)
            nc.vector.tensor_copy(out_sb[:], ps[:])
            nc.sync.dma_start(out=o[:], in_=out_sb[:])

    # Run the Bacc compile pipeline (wait splitting, reg alloc) before the
    # BIR is serialized for execution.
    nc.finalize()
    return nc


def kernel(input_in, input_out, flip):
    _ensure_path()
    from concourse.bass_utils import run_bass_kernel_spmd

    a = np.asarray(input_in, dtype=np.float32)
    b = np.asarray(input_out, dtype=np.float32)
    assert a.shape == (N, D) and b.shape == (N, D)

    nc = _cache.get("nc")
    if nc is None:
        nc = build()
        _cache["nc"] = nc

    in_maps = [
        {
            "input_in": np.ascontiguousarray(a[c * ROWS:(c + 1) * ROWS]).reshape(P, FREE),
            "input_out": np.ascontiguousarray(b[c * ROWS:(c + 1) * ROWS]).reshape(P, FREE),
        }
        for c in range(M)
    ]

    kw = {"trace": True} if TRACE else {}
    res = run_bass_kernel_spmd(nc, in_maps, core_ids=list(range(M)), **kw)
    LAST["results"] = res

    total = float(np.sum([r["partial"].astype(np.float64).sum() for r in res.results]))
    mean_sim = total / float(N)
    if int(np.asarray(flip)) != 0:
        val = mean_sim + 1.0
    else:
        val = 1.0 - mean_sim
    return np.array(val, dtype=np.float32)


# revision 13
# speedup vs baseline: 1.0773x; 1.0773x over previous
"""Trainium2 Bass kernel for nn_CLoss_inout: mean(1 - rowwise_dot(A, B)).

Full inputs A, B are [1048576, 128] f32. result = 1 - sum(A*B)/N (or
mean(A*B)+1 when flip). Data-parallel over 8 NeuronCores: core c gets rows
[c*131072, (c+1)*131072), viewed as [128 partitions x 131072 free] (order
of summation is irrelevant). Per tile of [128 x FT]: two HWDGE DMA loads,
one DVE tensor_mul (f32 inputs, bf16 product), and FT/512 PE matmuls
against a ones[128,1] stationary vector that accumulate per-column sums
into a single PSUM bank across the whole kernel. Tail: PSUM -> SBUF copy,
DMA the [1,512] partial out. The 8 per-core partials are summed on host.
"""

import numpy as np

N, D = 1048576, 128
M = 8                     # cores
ROWS = N // M             # 131072 rows per core
P = 128                   # SBUF partitions
FREE = ROWS * D // P      # 131072 f32 per partition per tensor
FT = 8192                 # tile free size: 128 x 8192 f32 = 4 MiB per DMA
BUFS = 2
MMF = 512                 # matmul moving free dim (one PSUM bank of f32)

TRACE = False             # test.py sets True to capture an NTFF profile
LAST = {}                 # stash of the most recent BassKernelResults

_cache = {}


def _ensure_path():
    import sys
    try:
        import concourse.bass  # noqa: F401
    except ImportError:
        sys.path.insert(0, "/opt/trn_rl_repo")


def build(free=FREE, ft=FT, bufs=BUFS):
    _ensure_path()
    import concourse.bacc as bacc
    import concourse.mybir as mybir
    from concourse.tile import TileContext

    assert free % ft == 0 and ft % MMF == 0
    nt = free // ft
    nj = ft // MMF
    # Bacc (not raw Bass): its compile pipeline splits multi-wait
    # instructions (TRN2 allows at most one sync wait per instruction).
    nc = bacc.Bacc(None, name="closs_inout")
    a = nc.dram_tensor("input_in", [P, free], mybir.dt.float32, kind="ExternalInput")
    b = nc.dram_tensor("input_out", [P, free], mybir.dt.float32, kind="ExternalInput")
    o = nc.dram_tensor("partial", [1, MMF], mybir.dt.float32, kind="ExternalOutput")

    with TileContext(nc) as tc:
        with (
            tc.tile_pool(name="pa", bufs=bufs) as pa,
            tc.tile_pool(name="pb", bufs=bufs) as pb,
            tc.tile_pool(name="pp", bufs=bufs) as pp,
            tc.tile_pool(name="misc", bufs=1) as misc,
            tc.tile_pool(name="psum", bufs=1, space="PSUM") as psum,
        ):
            ones = misc.tile([P, 1], mybir.dt.bfloat16)
            nc.gpsimd.memset(ones[:], 1.0)
            ps = psum.tile([1, MMF], mybir.dt.float32)
            for i in range(nt):
                at = pa.tile([P, ft], mybir.dt.float32)
                bt = pb.tile([P, ft], mybir.dt.float32)
                # Two physical HWDGE rings (SP + ACT): A-loads and B-loads
                # proceed in parallel instead of serializing on one FIFO.
                nc.sync.dma_start(out=at[:], in_=a[:, i * ft:(i + 1) * ft])
                nc.scalar.dma_start(out=bt[:], in_=b[:, i * ft:(i + 1) * ft])
                pt = pp.tile([P, ft], mybir.dt.bfloat16)
                nc.vector.tensor_mul(pt[:], at[:], bt[:])
                for j in range(nj):
                    # ps[0, n] += sum_p pt[p, j*MMF + n]
                    nc.tensor.matmul(
                        ps[:, :],
                        ones[:],
                        pt[:, j * MMF:(j + 1) * MMF],
                        start=(i == 0 and j == 0),
                        stop=(i == nt - 1 and j == nj - 1),
                    )
            out_sb = misc.tile([1, MMF], mybir.dt.float32)
            nc.vector.tensor_copy(out_sb[:], ps[:])
            nc.sync.dma_start(out=o[:], in_=out_sb[:])

    # Run the Bacc compile pipeline (wait splitting, reg alloc) before the
    # BIR is serialized for execution.
    nc.finalize()
    return nc


def kernel(input_in, input_out, flip):
    _ensure_path()
    from concourse.bass_utils import run_bass_kernel_spmd

    a = np.asarray(input_in, dtype=np.float32)
    b = np.asarray(input_out, dtype=np.float32)
    assert a.shape == (N, D) and b.shape == (N, D)

    nc = _cache.get("nc")
    if nc is None:
        nc = build()
        _cache["nc"] = nc

    in_maps = [
        {
            "input_in": np.ascontiguousarray(a[c * ROWS:(c + 1) * ROWS]).reshape(P, FREE),
            "input_out": np.ascontiguousarray(b[c * ROWS:(c + 1) * ROWS]).reshape(P, FREE),
        }
        for c in range(M)
    ]

    kw = {"trace": True} if TRACE else {}
    res = run_bass_kernel_spmd(nc, in_maps, core_ids=list(range(M)), **kw)
    LAST["results"] = res

    total = float(np.sum([r["partial"].astype(np.float64).sum() for r in res.results]))
    mean_sim = total / float(N)
    if int(np.asarray(flip)) != 0:
        val = mean_sim + 1.0
    else:
        val = 1.0 - mean_sim
    return np.array(val, dtype=np.float32)
